# revision 26
# baseline (speedup 1.0000x reference)
"""BEiT-style windowed attention block on 8 TRN2 NeuronCores.

Data-parallel over batch: each core handles 8 of the 64 batch elements.
Device kernel (per core), all matmul compute in fp16 with fp32 PSUM accum:

  1. qkT = (Wqk*s) @ x^T + bias   -> [1536 ch, 1576 tok] channel-major (q,k)
  2. v   = x @ Wv^T + vbias       -> token-major, written into an extended
     layout [v_even|1|0_63|1|v_odd] per head-pair so the PV matmul emits both
     the transposed attention output and the softmax denominators.
  3. scores^T per (batch, head-pair): row-packed pair of K=64 matmuls
     (tile_position row groups), exp on ACT (no max subtraction: logits are
     provably tiny), multiplied by host-precomputed exp(rel_pos_bias)^T.
  4. PV: out^T accumulated over key tiles; sums row embedded via the ones
     column; reciprocal + gpsimd partition_broadcast + DVE multiply
     normalizes straight into the proj lhsT layout.
  5. proj matmul + bias -> fp32 out.
"""

import sys

for _p in ("/opt/trn_rl_repo",):
    if _p not in sys.path:
        sys.path.insert(0, _p)

import numpy as np

import concourse.bass as bass
import concourse.mybir as mybir
import concourse.tile as tile
from concourse.bass_utils import run_bass_kernel_spmd
from concourse.vector_clock import ScopedClock

# ---------------------------------------------------------------- constants
DIM = 768
NUM_HEADS = 12
WH, WW = 14, 14
N_TOK = WH * WW + 1  # 197
NUM_REL_DIST = (2 * WH - 1) * (2 * WW - 1) + 3  # 732
HEAD_DIM = DIM // NUM_HEADS  # 64
SCALE = HEAD_DIM ** -0.5
B = 64
N_CORES = 8
B_LOC = B // N_CORES  # 8
TOK = B_LOC * N_TOK  # 1576
NPAIR = NUM_HEADS // 2  # 6
KT = [(0, 128), (128, 69)]  # key tiles within a batch
F16 = mybir.dt.float16
F32 = mybir.dt.float32

# ------------------------------------------------- walrus 1-wait workaround
# This walrus build rejects instructions carrying more than one semaphore
# wait ("Too many sync wait commands").  Split extra waits onto same-engine
# NOPs emitted immediately before the instruction during Tile lowering, and
# do the same for the kernel-tail drain's global-clock waits.
_MAXW = 1
_orig_commit_and_lower = tile.TileContext._commit_and_lower


def _patched_commit_and_lower(self, inst, original_block, old_bb_map, bb_to_exit_bb):
    si = inst.sync_info
    if si is not None and si.on_wait is not None and len(si.on_wait) > _MAXW:
        waits = list(si.on_wait)
        for w in waits[:-_MAXW]:
            nop = self.nc.engines[inst.engine].nop(nofuse=True)
            nop.ins.sync_info = mybir.SyncInfo(on_wait=[w], on_update=[])
        inst.sync_info = mybir.SyncInfo(
            on_wait=waits[-_MAXW:], on_update=list(si.on_update or [])
        )
    _orig_commit_and_lower(self, inst, original_block, old_bb_map, bb_to_exit_bb)


def _patched_drain_and_barrier(self, tick_clock, wait_clock):
    nc = self.nc
    probe = nc.sync.nop(nofuse=True)
    wait_clock.add_sem_waits(probe.ins, ScopedClock({None: tick_clock.global_clock}))
    si = probe.ins.sync_info
    waits = list(si.on_wait) if si is not None else []
    if len(waits) > _MAXW:
        probe.ins.sync_info = mybir.SyncInfo(on_wait=waits[:_MAXW], on_update=[])
        for i in range(_MAXW, len(waits), _MAXW):
            extra = nc.sync.nop(nofuse=True)
            extra.ins.sync_info = mybir.SyncInfo(
                on_wait=waits[i : i + _MAXW], on_update=[]
            )
    nc.sync.drain()
    nc.all_engine_barrier()
    assert self.sems is not None
    popped = nc._tile_sem_poison_stack.pop()
    assert popped is self._sem_poison
    nc.clear_and_free_semaphores(list(self.sems.allocated().values()))
    nc.all_engine_barrier()


def _act_recip_lnexp(nc, out, in_):
    # 1/s computed as exp(-ln(s)) on the Scalar engine.  ln and exp live in
    # the same activation table (natural_log_exp_and_others) so this costs
    # no ACT_TABLE_LOAD swaps, unlike the Reciprocal table op; and the DVE
    # InstReciprocal runs at ~6.6 ns/element on one partition (125 us total
    # here).  Inputs are softmax sums in [~50, 4000]: ln/exp tables are
    # accurate to ~1e-4 there, well inside tolerance.
    nc.scalar.activation(out, in_, mybir.ActivationFunctionType.Ln)
    nc.scalar.activation(out, out, mybir.ActivationFunctionType.Exp, scale=-1.0)


def _install_patches():
    tile.TileContext._commit_and_lower = _patched_commit_and_lower
    tile.TileContext._drain_and_barrier = _patched_drain_and_barrier


# ---------------------------------------------------------------- host prep
def _relative_position_index():
    coords = np.stack(np.meshgrid(np.arange(WH), np.arange(WW), indexing="ij"))
    cf = coords.reshape(2, -1)
    rel = cf[:, :, None] - cf[:, None, :]
    rel = rel.transpose(1, 2, 0).astype(np.int64)
    rel[:, :, 0] += WH - 1
    rel[:, :, 1] += WW - 1
    rel[:, :, 0] *= 2 * WW - 1
    idx = np.zeros((N_TOK, N_TOK), dtype=np.int64)
    idx[1:, 1:] = rel.sum(-1)
    idx[0, 0:] = NUM_REL_DIST - 3
    idx[0:, 0] = NUM_REL_DIST - 2
    idx[0, 0] = NUM_REL_DIST - 1
    return idx


def _host_prepare(x, qkv_weight, q_bias, v_bias, rel_pos_bias_table, proj_weight,
                  proj_bias):
    wqk = qkv_weight[: 2 * DIM].astype(np.float32).copy()
    wqk[:DIM] *= SCALE
    wqkT = np.ascontiguousarray(wqk.T).astype(np.float16)  # [768, 1536]
    wvT = np.ascontiguousarray(qkv_weight[2 * DIM :].T).astype(np.float16)
    wprojT = np.ascontiguousarray(proj_weight.T).astype(np.float16)  # [in, out]
    qkb = np.ascontiguousarray(
        (q_bias.astype(np.float32) * SCALE).reshape(6, 128).T
    ).astype(np.float32)  # [128, 6]
    vb = v_bias.astype(np.float16)
    pb = proj_bias.astype(np.float32)

    idx = _relative_position_index()
    bias_qk = rel_pos_bias_table.astype(np.float32)[idx]  # [q, k, h]
    e = np.exp(bias_qk).transpose(1, 2, 0)  # [k, h, q]
    e = np.ascontiguousarray(e.reshape(N_TOK, NPAIR, 2 * N_TOK)).astype(np.float16)
    rpb0 = np.ascontiguousarray(e[:128])  # [128, 6, 394]
    rpb1 = np.ascontiguousarray(e[128:])  # [69, 6, 394]

    shared = dict(wqkT=wqkT, wvT=wvT, wprojT=wprojT, qkb=qkb, vbias=vb,
                  pbias=pb, rpb0=rpb0, rpb1=rpb1)
    in_maps = []
    for c in range(N_CORES):
        xc = x[c * B_LOC : (c + 1) * B_LOC].reshape(TOK, DIM).astype(np.float16)
        xT = np.ascontiguousarray(xc.T)  # [768, 1576]
        in_maps.append(dict(xT=xT, **shared))
    return in_maps


# ------------------------------------------------------------- device build
def build_nc(phases=4, sub=9):
    _install_patches()
    nc = bass.Bass("TRN2", target_bir_lowering=False, debug=False,
                   num_devices=N_CORES)

    xT = nc.dram_tensor("xT", [DIM, TOK], F16, kind="ExternalInput")
    wqkT = nc.dram_tensor("wqkT", [DIM, 2 * DIM], F16, kind="ExternalInput")
    wvT = nc.dram_tensor("wvT", [DIM, DIM], F16, kind="ExternalInput")
    wprojT = nc.dram_tensor("wprojT", [DIM, DIM], F16, kind="ExternalInput")
    qkb = nc.dram_tensor("qkb", [128, 6], F32, kind="ExternalInput")
    vbias = nc.dram_tensor("vbias", [DIM], F16, kind="ExternalInput")
    pbias = nc.dram_tensor("pbias", [DIM], F32, kind="ExternalInput")
    rpb0 = nc.dram_tensor("rpb0", [128, NPAIR, 2 * N_TOK], F16, kind="ExternalInput")
    rpb1 = nc.dram_tensor("rpb1", [69, NPAIR, 2 * N_TOK], F16, kind="ExternalInput")
    out = nc.dram_tensor("out", [TOK, DIM], F32, kind="ExternalOutput")

    def bcast_ap(handle, n):
        ap = handle.ap()
        return bass.AP(tensor=ap.tensor, offset=ap.offset,
                       ap=[[0, 128]] + list(ap.ap))

    with tile.TileContext(nc) as tc:
        with (
            tc.tile_pool(name="const", bufs=1) as const,
            tc.tile_pool(name="exp", bufs=6) as exp_pool,
            tc.tile_pool(name="attn", bufs=3) as attn_pool,
            tc.tile_pool(name="bc", bufs=6) as bc_pool,
            tc.tile_pool(name="outp", bufs=3) as out_pool,
            tc.tile_pool(name="rc", bufs=3) as rc_pool,
            tc.tile_pool(name="un", bufs=4) as un_pool,
            tc.tile_pool(name="dscr", bufs=6, space="DRAM") as dscr_pool,
            tc.tile_pool(name="psA", bufs=4, space="PSUM") as psum,
            tc.tile_pool(name="psB", bufs=2, space="PSUM") as psum_pv,
        ):
            def rep_rows(row_ap, n):
                # replicate a [1, F] SBUF row across n partitions (step-0
                # partition dim) -- DMA source AP
                return bass.AP(tensor=row_ap.tensor, offset=row_ap.offset,
                               ap=[[0, n]] + list(row_ap.ap)[1:])

            # ---- constants into SBUF
            xT_sb = const.tile([128, 6, TOK], F16, tag="xT")
            xT_r = xT.ap().rearrange("(a p) n -> p a n", p=128)
            wqkT_sb = const.tile([128, 6, 2 * DIM], F16, tag="wqkT")
            wqk_r = wqkT.ap().rearrange("(a p) n -> p a n", p=128)
            for k in range(6):
                nc.sync.dma_start(wqkT_sb[:, k, :], wqk_r[:, k, :])
                nc.sync.dma_start(xT_sb[:, k, :], xT_r[:, k, :])
            wvT_sb = const.tile([128, 6, DIM], F16, tag="wvT")
            nc.sync.dma_start(wvT_sb[:], wvT.ap().rearrange("(a p) n -> p a n", p=128))
            wprojT_sb = const.tile([128, 6, DIM], F16, tag="wprojT")
            nc.sync.dma_start(wprojT_sb[:], wprojT.ap().rearrange("(a p) n -> p a n", p=128))
            qkb_sb = const.tile([128, 6], F32, tag="qkb")
            nc.sync.dma_start(qkb_sb[:], qkb.ap())
            vb_sb = const.tile([128, DIM], F16, tag="vb")
            nc.gpsimd.dma_start(vb_sb[:], bcast_ap(vbias, DIM))
            pb_sb = const.tile([128, DIM], F32, tag="pb")
            nc.gpsimd.dma_start(pb_sb[:], bcast_ap(pbias, DIM))
            rpb0_sb = const.tile([128, NPAIR, 2 * N_TOK], F16, tag="rpb0")
            nc.sync.dma_start(rpb0_sb[:], rpb0.ap())
            rpb1_sb = const.tile([69, NPAIR, 2 * N_TOK], F16, tag="rpb1")
            nc.sync.dma_start(rpb1_sb[:], rpb1.ap())

            kT_sb = const.tile([128, 6, TOK], F16, tag="kT")
            # q in zero-padded head slots: slot (c, j) holds head 2c+j on
            # partitions 64j:64j+64, zeros elsewhere, so QK^T runs as a
            # plain K=128 matmul against the packed k chunk.  (Row-group
            # packed K=64 matmul pairs crash this runtime.)
            qz_sb = const.tile([128, 6, 2, TOK], F16, tag="qz")
            nc.gpsimd.memset(qz_sb[0:64, :, 1, :], 0.0)
            nc.gpsimd.memset(qz_sb[64:128, :, 0, :], 0.0)
            # v extended layout per (token-tile, pair):
            #   [0:64]=v_even [64]=1 | odd block (65+): [0:32]=0 [32]=1
            #   [33:64]=0 [64:128]=v_odd  -> odd sums land on psum row 32
            vext_sb = const.tile([128, 2 * B_LOC, NPAIR, 193], F16, tag="vext")
            nc.gpsimd.memset(vext_sb[:, :, :, 65:97], 0.0)
            nc.gpsimd.memset(vext_sb[:, :, :, 98:129], 0.0)
            nc.vector.memset(vext_sb[:, :, :, 64:65], 1.0)
            nc.vector.memset(vext_sb[:, :, :, 97:98], 1.0)

            # ---- phase 1: qkT [ch, tok] = Wqk' @ x^T (+ scaled q bias)
            NCH = 394  # 2 batches of queries per psum chunk
            for m in range(12):
                for nch in range(4):
                    ps = psum.tile([128, NCH], F32, tag="ps")
                    for k in range(6):
                        nc.tensor.matmul(
                            ps[:],
                            lhsT=wqkT_sb[:, k, m * 128 : (m + 1) * 128],
                            rhs=xT_sb[:, k, nch * NCH : (nch + 1) * NCH],
                            start=(k == 0), stop=(k == 5),
                        )
                    cols = slice(nch * NCH, (nch + 1) * NCH)
                    if m < 6:
                        nc.scalar.activation(
                            qz_sb[0:64, m, 0, cols], ps[0:64],
                            mybir.ActivationFunctionType.Identity,
                            bias=qkb_sb[0:64, m : m + 1],
                        )
                        nc.scalar.activation(
                            qz_sb[64:128, m, 1, cols], ps[64:128],
                            mybir.ActivationFunctionType.Identity,
                            bias=qkb_sb[64:128, m : m + 1],
                        )
                    else:
                        nc.scalar.activation(
                            kT_sb[:, m - 6, cols], ps[:],
                            mybir.ActivationFunctionType.Copy)

            # ---- phase 2: v (token-major) into vext (+ v bias)
            for bt in range(2 * B_LOC):
                b, t = divmod(bt, 2)
                tbase, tsz = KT[t]
                col0 = b * N_TOK + tbase
                for ncb in range(2):
                    ps = psum.tile([128, 384], F32, tag="ps")
                    for k in range(6):
                        nc.tensor.matmul(
                            ps[:tsz],
                            lhsT=xT_sb[:, k, col0 : col0 + tsz],
                            rhs=wvT_sb[:, k, ncb * 384 : (ncb + 1) * 384],
                            start=(k == 0), stop=(k == 5),
                        )
                    src = ps[:tsz].rearrange("p (c j d) -> p c j d", c=3, j=2)
                    vbv = vb_sb[:tsz, ncb * 384 : (ncb + 1) * 384].rearrange(
                        "p (c j d) -> p c j d", c=3, j=2)
                    pear = vext_sb[:tsz, bt, 3 * ncb : 3 * ncb + 3, :]
                    nc.vector.tensor_add(
                        out=pear[:, :, 0:64], in0=src[:, :, 0, :], in1=vbv[:, :, 0, :])
                    nc.vector.tensor_add(
                        out=pear[:, :, 129:193], in0=src[:, :, 1, :], in1=vbv[:, :, 1, :])

            # ---- phase 3: attention per (batch, pair-group of 2 head-pairs)
            for b in range(B_LOC if phases >= 3 else 0):
                q0 = b * N_TOK
                attn_sb = attn_pool.tile([128, 6, N_TOK], F16, tag="attn")
                for g in range(NPAIR // 2):
                    # PV psum for 2 pairs; 256-stride keeps each matmul
                    # region inside one PSUM bank.
                    pvg = psum_pv.tile([128, 2, 2, 256], F32, tag="pvg")
                    for pig in (0, 1):
                        c = 2 * g + pig
                        # scores^T + exp + rel-pos multiplier, per key tile
                        ets = []
                        for t, (kbase, ksz) in enumerate(KT):
                            kcol = q0 + kbase
                            ps = psum.tile([128, 2 * N_TOK], F32, tag="ps")
                            nc.tensor.matmul(
                                ps[:ksz, :].rearrange("p (j q) -> p j q", j=2),
                                lhsT=kT_sb[:, c, kcol : kcol + ksz],
                                rhs=qz_sb[:, c, :, q0 : q0 + N_TOK],
                                start=True, stop=True,
                            )
                            et = exp_pool.tile([128, 2 * N_TOK], F16, tag="exp")
                            nc.scalar.activation(
                                et[:ksz], ps[:ksz], mybir.ActivationFunctionType.Exp)
                            rp = rpb0_sb if t == 0 else rpb1_sb
                            nc.vector.tensor_mul(et[:ksz], et[:ksz], rp[:ksz, c, :])
                            ets.append((et, ksz))

                        # PV (attention-out transposed + sums rows).  The
                        # even head uses the full 128-wide lhsT slice (cols
                        # 65..127 are zeros / the odd ones-column) so every
                        # psum row is written, letting the un-copy read the
                        # whole block.
                        for j in ((0, 1) if sub >= 2 else ()):
                            outap = pvg[:, pig, j, 0:N_TOK]
                            lo, hi = (0, 128) if j == 0 else (65, 193)
                            for t, (et, ksz) in enumerate(ets):
                                nc.tensor.matmul(
                                    outap,
                                    lhsT=vext_sb[:ksz, 2 * b + t, c, lo:hi],
                                    rhs=et[:ksz, j * N_TOK : (j + 1) * N_TOK],
                                    start=(t == 0), stop=(t == 1),
                                )

                    # softmax denominators sit at psum row 64 (even heads,
                    # j=0 block) and row 32 (odd heads, j=1 block); 1/s via
                    # ln+exp on ACT (same activation table as Exp).
                    recip = rc_pool.tile([128, 2, 2 * N_TOK], F32, tag="recip")
                    _act_recip_lnexp(
                        nc, recip[64:65, :, 0:N_TOK], pvg[64:65, :, 0, 0:N_TOK])
                    _act_recip_lnexp(
                        nc, recip[32:33, :, N_TOK : 2 * N_TOK],
                        pvg[32:33, :, 1, 0:N_TOK])
                    # bounce recip rows through DRAM (SBUF APs cannot have
                    # step-0 partition dims), then one broadcast-read per
                    # pair: partitions 0:64 <- even row, 64:128 <- odd row.
                    dscr = dscr_pool.tile([2, 2, N_TOK], F32, tag="dscr")
                    nc.sync.dma_start(dscr[0:1, :, :], recip[64:65, :, 0:N_TOK])
                    nc.sync.dma_start(
                        dscr[1:2, :, :], recip[32:33, :, N_TOK : 2 * N_TOK])
                    for pig in (0, 1):
                        c = 2 * g + pig
                        bc = bc_pool.tile([128, N_TOK], F32, tag="bc")
                        dr = dscr[:, pig, :]
                        src = bass.AP(
                            tensor=dr.tensor, offset=dr.offset,
                            ap=[list(dr.ap)[0], [0, 64], [1, N_TOK]])
                        nc.gpsimd.dma_start(bc[:], src)
                        nc.vector.tensor_mul(
                            attn_sb[0:64, c, :], pvg[0:64, pig, 0, 0:N_TOK],
                            bc[0:64, :])
                        nc.vector.tensor_mul(
                            attn_sb[64:128, c, :], pvg[64:128, pig, 1, 0:N_TOK],
                            bc[64:128, :])

                # ---- proj + bias for this batch
                for tbase, tsz in (KT if phases >= 4 else []):
                    osb = out_pool.tile([128, DIM], F32, tag="osb")
                    for ncb in range(2):
                        ps = psum.tile([128, 384], F32, tag="ps")
                        for k in range(6):
                            nc.tensor.matmul(
                                ps[:tsz],
                                lhsT=attn_sb[:, k, tbase : tbase + tsz],
                                rhs=wprojT_sb[:, k, ncb * 384 : (ncb + 1) * 384],
                                start=(k == 0), stop=(k == 5),
                            )
                        nc.vector.tensor_add(
                            out=osb[:tsz, ncb * 384 : (ncb + 1) * 384],
                            in0=ps[:tsz],
                            in1=pb_sb[:tsz, ncb * 384 : (ncb + 1) * 384],
                        )
                    nc.sync.dma_start(
                        out.ap()[q0 + tbase : q0 + tbase + tsz, :], osb[:tsz])
            if phases < 4:
                # debug: dump some qkT into out so the output is written
                dbg = out_pool.tile([128, DIM], F32, tag="osb")
                nc.scalar.activation(dbg[:, 0:DIM], kT_sb[:, 0, 0:DIM],
                                     mybir.ActivationFunctionType.Copy)
                nc.vector.tensor_add(dbg[:], dbg[:], vext_sb[:, 0, 0, 0:1].to_broadcast([128, DIM]))
                for r in range(0, TOK, 128):
                    sz = min(128, TOK - r)
                    nc.sync.dma_start(out.ap()[r : r + sz, :], dbg[:sz])
    return nc


_NC_CACHE = None


def _get_nc():
    global _NC_CACHE
    if _NC_CACHE is None:
        _NC_CACHE = build_nc()
    return _NC_CACHE


def _execute(inputs, trace=False):
    in_maps = _host_prepare(**inputs)
    nc = _get_nc()
    res = run_bass_kernel_spmd(nc, in_maps, core_ids=list(range(N_CORES)),
                               trace=trace)
    outs = [res.results[c]["out"].reshape(B_LOC, N_TOK, DIM) for c in range(N_CORES)]
    return np.concatenate(outs, axis=0), res


def kernel(**inputs) -> np.ndarray:
    out, _ = _execute(inputs, trace=False)
    return out


# revision 27
# speedup vs baseline: 1.1272x; 1.1272x over previous
"""BEiT-style windowed attention block on 8 TRN2 NeuronCores.

Data-parallel over batch: each core handles 8 of the 64 batch elements.
Device kernel (per core), all matmul compute in fp16 with fp32 PSUM accum:

  1. qkT = (Wqk*s) @ x^T + bias   -> [1536 ch, 1576 tok] channel-major (q,k)
  2. v   = x @ Wv^T + vbias       -> token-major, written into an extended
     layout [v_even|1|0_63|1|v_odd] per head-pair so the PV matmul emits both
     the transposed attention output and the softmax denominators.
  3. scores^T per (batch, head-pair): row-packed pair of K=64 matmuls
     (tile_position row groups), exp on ACT (no max subtraction: logits are
     provably tiny), multiplied by host-precomputed exp(rel_pos_bias)^T.
  4. PV: out^T accumulated over key tiles; sums row embedded via the ones
     column; reciprocal + gpsimd partition_broadcast + DVE multiply
     normalizes straight into the proj lhsT layout.
  5. proj matmul + bias -> fp32 out.
"""

import sys

for _p in ("/opt/trn_rl_repo",):
    if _p not in sys.path:
        sys.path.insert(0, _p)

import numpy as np

import concourse.bass as bass
import concourse.mybir as mybir
import concourse.tile as tile
from concourse.bass_utils import run_bass_kernel_spmd
from concourse.vector_clock import ScopedClock

# ---------------------------------------------------------------- constants
DIM = 768
NUM_HEADS = 12
WH, WW = 14, 14
N_TOK = WH * WW + 1  # 197
NUM_REL_DIST = (2 * WH - 1) * (2 * WW - 1) + 3  # 732
HEAD_DIM = DIM // NUM_HEADS  # 64
SCALE = HEAD_DIM ** -0.5
B = 64
N_CORES = 8
B_LOC = B // N_CORES  # 8
TOK = B_LOC * N_TOK  # 1576
NPAIR = NUM_HEADS // 2  # 6
KT = [(0, 128), (128, 69)]  # key tiles within a batch
F16 = mybir.dt.float16
F32 = mybir.dt.float32

# ------------------------------------------------- walrus 1-wait workaround
# This walrus build rejects instructions carrying more than one semaphore
# wait ("Too many sync wait commands").  Split extra waits onto same-engine
# NOPs emitted immediately before the instruction during Tile lowering, and
# do the same for the kernel-tail drain's global-clock waits.
_MAXW = 1
_orig_commit_and_lower = tile.TileContext._commit_and_lower


def _patched_commit_and_lower(self, inst, original_block, old_bb_map, bb_to_exit_bb):
    si = inst.sync_info
    if si is not None and si.on_wait is not None and len(si.on_wait) > _MAXW:
        waits = list(si.on_wait)
        for w in waits[:-_MAXW]:
            nop = self.nc.engines[inst.engine].nop(nofuse=True)
            nop.ins.sync_info = mybir.SyncInfo(on_wait=[w], on_update=[])
        inst.sync_info = mybir.SyncInfo(
            on_wait=waits[-_MAXW:], on_update=list(si.on_update or [])
        )
    _orig_commit_and_lower(self, inst, original_block, old_bb_map, bb_to_exit_bb)


def _patched_drain_and_barrier(self, tick_clock, wait_clock):
    nc = self.nc
    probe = nc.sync.nop(nofuse=True)
    wait_clock.add_sem_waits(probe.ins, ScopedClock({None: tick_clock.global_clock}))
    si = probe.ins.sync_info
    waits = list(si.on_wait) if si is not None else []
    if len(waits) > _MAXW:
        probe.ins.sync_info = mybir.SyncInfo(on_wait=waits[:_MAXW], on_update=[])
        for i in range(_MAXW, len(waits), _MAXW):
            extra = nc.sync.nop(nofuse=True)
            extra.ins.sync_info = mybir.SyncInfo(
                on_wait=waits[i : i + _MAXW], on_update=[]
            )
    nc.sync.drain()
    nc.all_engine_barrier()
    assert self.sems is not None
    popped = nc._tile_sem_poison_stack.pop()
    assert popped is self._sem_poison
    nc.clear_and_free_semaphores(list(self.sems.allocated().values()))
    nc.all_engine_barrier()


def _act_recip_lnexp(nc, out, in_):
    # 1/s computed as exp(-ln(s)) on the Scalar engine.  ln and exp live in
    # the same activation table (natural_log_exp_and_others) so this costs
    # no ACT_TABLE_LOAD swaps, unlike the Reciprocal table op; and the DVE
    # InstReciprocal runs at ~6.6 ns/element on one partition (125 us total
    # here).  Inputs are softmax sums in [~50, 4000]: ln/exp tables are
    # accurate to ~1e-4 there, well inside tolerance.
    nc.scalar.activation(out, in_, mybir.ActivationFunctionType.Ln)
    nc.scalar.activation(out, out, mybir.ActivationFunctionType.Exp, scale=-1.0)


def _install_patches():
    tile.TileContext._commit_and_lower = _patched_commit_and_lower
    tile.TileContext._drain_and_barrier = _patched_drain_and_barrier


# ---------------------------------------------------------------- host prep
def _relative_position_index():
    coords = np.stack(np.meshgrid(np.arange(WH), np.arange(WW), indexing="ij"))
    cf = coords.reshape(2, -1)
    rel = cf[:, :, None] - cf[:, None, :]
    rel = rel.transpose(1, 2, 0).astype(np.int64)
    rel[:, :, 0] += WH - 1
    rel[:, :, 1] += WW - 1
    rel[:, :, 0] *= 2 * WW - 1
    idx = np.zeros((N_TOK, N_TOK), dtype=np.int64)
    idx[1:, 1:] = rel.sum(-1)
    idx[0, 0:] = NUM_REL_DIST - 3
    idx[0:, 0] = NUM_REL_DIST - 2
    idx[0, 0] = NUM_REL_DIST - 1
    return idx


def _host_prepare(x, qkv_weight, q_bias, v_bias, rel_pos_bias_table, proj_weight,
                  proj_bias):
    wqk = qkv_weight[: 2 * DIM].astype(np.float32).copy()
    wqk[:DIM] *= SCALE
    wqkT = np.ascontiguousarray(wqk.T).astype(np.float16)  # [768, 1536]
    wvT = np.ascontiguousarray(qkv_weight[2 * DIM :].T).astype(np.float16)
    wprojT = np.ascontiguousarray(proj_weight.T).astype(np.float16)  # [in, out]
    qkb = np.ascontiguousarray(
        (q_bias.astype(np.float32) * SCALE).reshape(6, 128).T
    ).astype(np.float32)  # [128, 6]
    vb = v_bias.astype(np.float16)
    pb = proj_bias.astype(np.float32)

    idx = _relative_position_index()
    bias_qk = rel_pos_bias_table.astype(np.float32)[idx]  # [q, k, h]
    e = np.exp(bias_qk).transpose(1, 2, 0)  # [k, h, q]
    e = np.ascontiguousarray(e.reshape(N_TOK, NPAIR, 2 * N_TOK)).astype(np.float16)
    rpb0 = np.ascontiguousarray(e[:128])  # [128, 6, 394]
    rpb1 = np.ascontiguousarray(e[128:])  # [69, 6, 394]

    shared = dict(wqkT=wqkT, wvT=wvT, wprojT=wprojT, qkb=qkb, vbias=vb,
                  pbias=pb, rpb0=rpb0, rpb1=rpb1)
    in_maps = []
    for c in range(N_CORES):
        xc = x[c * B_LOC : (c + 1) * B_LOC].reshape(TOK, DIM).astype(np.float16)
        xT = np.ascontiguousarray(xc.T)  # [768, 1576]
        in_maps.append(dict(xT=xT, **shared))
    return in_maps


# ------------------------------------------------------------- device build
def build_nc(phases=4, sub=9):
    _install_patches()
    nc = bass.Bass("TRN2", target_bir_lowering=False, debug=False,
                   num_devices=N_CORES)

    xT = nc.dram_tensor("xT", [DIM, TOK], F16, kind="ExternalInput")
    wqkT = nc.dram_tensor("wqkT", [DIM, 2 * DIM], F16, kind="ExternalInput")
    wvT = nc.dram_tensor("wvT", [DIM, DIM], F16, kind="ExternalInput")
    wprojT = nc.dram_tensor("wprojT", [DIM, DIM], F16, kind="ExternalInput")
    qkb = nc.dram_tensor("qkb", [128, 6], F32, kind="ExternalInput")
    vbias = nc.dram_tensor("vbias", [DIM], F16, kind="ExternalInput")
    pbias = nc.dram_tensor("pbias", [DIM], F32, kind="ExternalInput")
    rpb0 = nc.dram_tensor("rpb0", [128, NPAIR, 2 * N_TOK], F16, kind="ExternalInput")
    rpb1 = nc.dram_tensor("rpb1", [69, NPAIR, 2 * N_TOK], F16, kind="ExternalInput")
    out = nc.dram_tensor("out", [TOK, DIM], F32, kind="ExternalOutput")

    def bcast_ap(handle, n):
        ap = handle.ap()
        return bass.AP(tensor=ap.tensor, offset=ap.offset,
                       ap=[[0, 128]] + list(ap.ap))

    with tile.TileContext(nc) as tc:
        with (
            tc.tile_pool(name="const", bufs=1) as const,
            tc.tile_pool(name="exp", bufs=6) as exp_pool,
            tc.tile_pool(name="attn", bufs=3) as attn_pool,
            tc.tile_pool(name="bc", bufs=6) as bc_pool,
            tc.tile_pool(name="outp", bufs=3) as out_pool,
            tc.tile_pool(name="rc", bufs=3) as rc_pool,
            tc.tile_pool(name="un", bufs=4) as un_pool,
            tc.tile_pool(name="dscr", bufs=6, space="DRAM") as dscr_pool,
            tc.tile_pool(name="psA", bufs=4, space="PSUM") as psum,
            tc.tile_pool(name="psB", bufs=2, space="PSUM") as psum_pv,
        ):
            def rep_rows(row_ap, n):
                # replicate a [1, F] SBUF row across n partitions (step-0
                # partition dim) -- DMA source AP
                return bass.AP(tensor=row_ap.tensor, offset=row_ap.offset,
                               ap=[[0, n]] + list(row_ap.ap)[1:])

            # ---- constants into SBUF
            xT_sb = const.tile([128, 6, TOK], F16, tag="xT")
            xT_r = xT.ap().rearrange("(a p) n -> p a n", p=128)
            wqkT_sb = const.tile([128, 6, 2 * DIM], F16, tag="wqkT")
            wqk_r = wqkT.ap().rearrange("(a p) n -> p a n", p=128)
            for k in range(6):
                nc.sync.dma_start(wqkT_sb[:, k, :], wqk_r[:, k, :])
                nc.sync.dma_start(xT_sb[:, k, :], xT_r[:, k, :])
            wvT_sb = const.tile([128, 6, DIM], F16, tag="wvT")
            nc.sync.dma_start(wvT_sb[:], wvT.ap().rearrange("(a p) n -> p a n", p=128))
            wprojT_sb = const.tile([128, 6, DIM], F16, tag="wprojT")
            nc.sync.dma_start(wprojT_sb[:], wprojT.ap().rearrange("(a p) n -> p a n", p=128))
            qkb_sb = const.tile([128, 6], F32, tag="qkb")
            nc.sync.dma_start(qkb_sb[:], qkb.ap())
            vb_sb = const.tile([128, DIM], F16, tag="vb")
            nc.gpsimd.dma_start(vb_sb[:], bcast_ap(vbias, DIM))
            pb_sb = const.tile([128, DIM], F32, tag="pb")
            nc.gpsimd.dma_start(pb_sb[:], bcast_ap(pbias, DIM))
            rpb0_sb = const.tile([128, NPAIR, 2 * N_TOK], F16, tag="rpb0")
            nc.sync.dma_start(rpb0_sb[:], rpb0.ap())
            rpb1_sb = const.tile([69, NPAIR, 2 * N_TOK], F16, tag="rpb1")
            nc.sync.dma_start(rpb1_sb[:], rpb1.ap())

            kT_sb = const.tile([128, 6, TOK], F16, tag="kT")
            # q in zero-padded head slots: slot (c, j) holds head 2c+j on
            # partitions 64j:64j+64, zeros elsewhere, so QK^T runs as a
            # plain K=128 matmul against the packed k chunk.  (Row-group
            # packed K=64 matmul pairs crash this runtime.)
            qz_sb = const.tile([128, 6, 2, TOK], F16, tag="qz")
            nc.gpsimd.memset(qz_sb[0:64, :, 1, :], 0.0)
            nc.gpsimd.memset(qz_sb[64:128, :, 0, :], 0.0)
            # v extended layout per (token-tile, pair):
            #   [0:64]=v_even [64]=1 | odd block (65+): [0:32]=0 [32]=1
            #   [33:64]=0 [64:128]=v_odd  -> odd sums land on psum row 32
            vext_sb = const.tile([128, 2 * B_LOC, NPAIR, 193], F16, tag="vext")
            nc.gpsimd.memset(vext_sb[:, :, :, 65:97], 0.0)
            nc.gpsimd.memset(vext_sb[:, :, :, 98:129], 0.0)
            nc.vector.memset(vext_sb[:, :, :, 64:65], 1.0)
            nc.vector.memset(vext_sb[:, :, :, 97:98], 1.0)

            # ---- phase 1: qkT [ch, tok] = Wqk' @ x^T (+ scaled q bias)
            NCH = 394  # 2 batches of queries per psum chunk
            for m in range(12):
                for nch in range(4):
                    ps = psum.tile([128, NCH], F32, tag="ps")
                    for k in range(6):
                        nc.tensor.matmul(
                            ps[:],
                            lhsT=wqkT_sb[:, k, m * 128 : (m + 1) * 128],
                            rhs=xT_sb[:, k, nch * NCH : (nch + 1) * NCH],
                            start=(k == 0), stop=(k == 5),
                        )
                    cols = slice(nch * NCH, (nch + 1) * NCH)
                    if m < 6:
                        nc.scalar.activation(
                            qz_sb[0:64, m, 0, cols], ps[0:64],
                            mybir.ActivationFunctionType.Identity,
                            bias=qkb_sb[0:64, m : m + 1],
                        )
                        nc.scalar.activation(
                            qz_sb[64:128, m, 1, cols], ps[64:128],
                            mybir.ActivationFunctionType.Identity,
                            bias=qkb_sb[64:128, m : m + 1],
                        )
                    else:
                        nc.scalar.activation(
                            kT_sb[:, m - 6, cols], ps[:],
                            mybir.ActivationFunctionType.Copy)

            # ---- phase 2: v (token-major) into vext (+ v bias)
            for bt in range(2 * B_LOC):
                b, t = divmod(bt, 2)
                tbase, tsz = KT[t]
                col0 = b * N_TOK + tbase
                for ncb in range(2):
                    ps = psum.tile([128, 384], F32, tag="ps")
                    for k in range(6):
                        nc.tensor.matmul(
                            ps[:tsz],
                            lhsT=xT_sb[:, k, col0 : col0 + tsz],
                            rhs=wvT_sb[:, k, ncb * 384 : (ncb + 1) * 384],
                            start=(k == 0), stop=(k == 5),
                        )
                    src = ps[:tsz].rearrange("p (c j d) -> p c j d", c=3, j=2)
                    vbv = vb_sb[:tsz, ncb * 384 : (ncb + 1) * 384].rearrange(
                        "p (c j d) -> p c j d", c=3, j=2)
                    pear = vext_sb[:tsz, bt, 3 * ncb : 3 * ncb + 3, :]
                    nc.vector.tensor_add(
                        out=pear[:, :, 0:64], in0=src[:, :, 0, :], in1=vbv[:, :, 0, :])
                    nc.vector.tensor_add(
                        out=pear[:, :, 129:193], in0=src[:, :, 1, :], in1=vbv[:, :, 1, :])

            # ---- phase 3: attention per (batch, pair-group of 2 head-pairs)
            for b in range(B_LOC if phases >= 3 else 0):
                q0 = b * N_TOK
                attn_sb = attn_pool.tile([128, 6, N_TOK], F16, tag="attn")
                for g in range(NPAIR // 2):
                    # PV psum for 2 pairs; 256-stride keeps each matmul
                    # region inside one PSUM bank.
                    pvg = psum_pv.tile([128, 2, 2, 256], F32, tag="pvg")
                    for pig in (0, 1):
                        c = 2 * g + pig
                        # scores^T + exp + rel-pos multiplier, per key tile
                        ets = []
                        for t, (kbase, ksz) in enumerate(KT):
                            kcol = q0 + kbase
                            ps = psum.tile([128, 2 * N_TOK], F32, tag="ps")
                            nc.tensor.matmul(
                                ps[:ksz, :].rearrange("p (j q) -> p j q", j=2),
                                lhsT=kT_sb[:, c, kcol : kcol + ksz],
                                rhs=qz_sb[:, c, :, q0 : q0 + N_TOK],
                                start=True, stop=True,
                            )
                            et = exp_pool.tile([128, 2 * N_TOK], F16, tag="exp")
                            nc.scalar.activation(
                                et[:ksz], ps[:ksz], mybir.ActivationFunctionType.Exp)
                            rp = rpb0_sb if t == 0 else rpb1_sb
                            nc.vector.tensor_mul(et[:ksz], et[:ksz], rp[:ksz, c, :])
                            ets.append((et, ksz))

                        # PV (attention-out transposed + sums rows)
                        for j in ((0, 1) if sub >= 2 else ()):
                            outap = (pvg[0:65, pig, 0, 0:N_TOK] if j == 0
                                     else pvg[:, pig, 1, 0:N_TOK])
                            lo, hi = (0, 65) if j == 0 else (65, 193)
                            for t, (et, ksz) in enumerate(ets):
                                nc.tensor.matmul(
                                    outap,
                                    lhsT=vext_sb[:ksz, 2 * b + t, c, lo:hi],
                                    rhs=et[:ksz, j * N_TOK : (j + 1) * N_TOK],
                                    start=(t == 0), stop=(t == 1),
                                )

                    # softmax denominators sit at psum row 64 (even heads,
                    # j=0 block) and row 32 (odd heads, j=1 block); 1/s via
                    # ln+exp on ACT (same activation table as Exp).
                    recip = rc_pool.tile([128, 2, 2 * N_TOK], F32, tag="recip")
                    _act_recip_lnexp(
                        nc, recip[64:65, :, 0:N_TOK], pvg[64:65, :, 0, 0:N_TOK])
                    _act_recip_lnexp(
                        nc, recip[32:33, :, N_TOK : 2 * N_TOK],
                        pvg[32:33, :, 1, 0:N_TOK])
                    # bounce recip rows through DRAM (SBUF APs cannot have
                    # step-0 partition dims), then one broadcast-read per
                    # pair: partitions 0:64 <- even row, 64:128 <- odd row.
                    dscr = dscr_pool.tile([2, 2, N_TOK], F32, tag="dscr")
                    nc.sync.dma_start(dscr[0:1, :, :], recip[64:65, :, 0:N_TOK])
                    nc.sync.dma_start(
                        dscr[1:2, :, :], recip[32:33, :, N_TOK : 2 * N_TOK])
                    for pig in (0, 1):
                        c = 2 * g + pig
                        bc = bc_pool.tile([128, N_TOK], F32, tag="bc")
                        nc.gpsimd.dma_start(
                            bc[0:64, :], rep_rows(dscr[0:1, pig, :], 64))
                        nc.gpsimd.dma_start(
                            bc[64:128, :], rep_rows(dscr[1:2, pig, :], 64))
                        nc.vector.tensor_mul(
                            attn_sb[0:64, c, :], pvg[0:64, pig, 0, 0:N_TOK],
                            bc[0:64, :])
                        nc.vector.tensor_mul(
                            attn_sb[64:128, c, :], pvg[64:128, pig, 1, 0:N_TOK],
                            bc[64:128, :])

                # ---- proj + bias for this batch
                for tbase, tsz in (KT if phases >= 4 else []):
                    osb = out_pool.tile([128, DIM], F32, tag="osb")
                    for ncb in range(2):
                        ps = psum.tile([128, 384], F32, tag="ps")
                        for k in range(6):
                            nc.tensor.matmul(
                                ps[:tsz],
                                lhsT=attn_sb[:, k, tbase : tbase + tsz],
                                rhs=wprojT_sb[:, k, ncb * 384 : (ncb + 1) * 384],
                                start=(k == 0), stop=(k == 5),
                            )
                        nc.vector.tensor_add(
                            out=osb[:tsz, ncb * 384 : (ncb + 1) * 384],
                            in0=ps[:tsz],
                            in1=pb_sb[:tsz, ncb * 384 : (ncb + 1) * 384],
                        )
                    nc.sync.dma_start(
                        out.ap()[q0 + tbase : q0 + tbase + tsz, :], osb[:tsz])
            if phases < 4:
                # debug: dump some qkT into out so the output is written
                dbg = out_pool.tile([128, DIM], F32, tag="osb")
                nc.scalar.activation(dbg[:, 0:DIM], kT_sb[:, 0, 0:DIM],
                                     mybir.ActivationFunctionType.Copy)
                nc.vector.tensor_add(dbg[:], dbg[:], vext_sb[:, 0, 0, 0:1].to_broadcast([128, DIM]))
                for r in range(0, TOK, 128):
                    sz = min(128, TOK - r)
                    nc.sync.dma_start(out.ap()[r : r + sz, :], dbg[:sz])
    return nc


_NC_CACHE = None


def _get_nc():
    global _NC_CACHE
    if _NC_CACHE is None:
        _NC_CACHE = build_nc()
    return _NC_CACHE


def _execute(inputs, trace=False):
    in_maps = _host_prepare(**inputs)
    nc = _get_nc()
    res = run_bass_kernel_spmd(nc, in_maps, core_ids=list(range(N_CORES)),
                               trace=trace)
    outs = [res.results[c]["out"].reshape(B_LOC, N_TOK, DIM) for c in range(N_CORES)]
    return np.concatenate(outs, axis=0), res


def kernel(**inputs) -> np.ndarray:
    out, _ = _execute(inputs, trace=False)
    return out


# revision 28
# speedup vs baseline: 1.2686x; 1.1255x over previous
"""BEiT-style windowed attention block on 8 TRN2 NeuronCores.

Data-parallel over batch: each core handles 8 of the 64 batch elements.
Device kernel (per core), all matmul compute in fp16 with fp32 PSUM accum:

  1. qkT = (Wqk*s) @ x^T + bias   -> [1536 ch, 1576 tok] channel-major (q,k)
  2. v   = x @ Wv^T + vbias       -> token-major, written into an extended
     layout [v_even|1|0_63|1|v_odd] per head-pair so the PV matmul emits both
     the transposed attention output and the softmax denominators.
  3. scores^T per (batch, head-pair): row-packed pair of K=64 matmuls
     (tile_position row groups), exp on ACT (no max subtraction: logits are
     provably tiny), multiplied by host-precomputed exp(rel_pos_bias)^T.
  4. PV: out^T accumulated over key tiles; sums row embedded via the ones
     column; reciprocal + gpsimd partition_broadcast + DVE multiply
     normalizes straight into the proj lhsT layout.
  5. proj matmul + bias -> fp32 out.
"""

import sys

for _p in ("/opt/trn_rl_repo",):
    if _p not in sys.path:
        sys.path.insert(0, _p)

import numpy as np

import concourse.bass as bass
import concourse.mybir as mybir
import concourse.tile as tile
from concourse.bass_utils import run_bass_kernel_spmd
from concourse.vector_clock import ScopedClock

# ---------------------------------------------------------------- constants
DIM = 768
NUM_HEADS = 12
WH, WW = 14, 14
N_TOK = WH * WW + 1  # 197
NUM_REL_DIST = (2 * WH - 1) * (2 * WW - 1) + 3  # 732
HEAD_DIM = DIM // NUM_HEADS  # 64
SCALE = HEAD_DIM ** -0.5
B = 64
N_CORES = 8
B_LOC = B // N_CORES  # 8
TOK = B_LOC * N_TOK  # 1576
NPAIR = NUM_HEADS // 2  # 6
KT = [(0, 128), (128, 69)]  # key tiles within a batch
F16 = mybir.dt.float16
F32 = mybir.dt.float32

# ------------------------------------------------- walrus 1-wait workaround
# This walrus build rejects instructions carrying more than one semaphore
# wait ("Too many sync wait commands").  Split extra waits onto same-engine
# NOPs emitted immediately before the instruction during Tile lowering, and
# do the same for the kernel-tail drain's global-clock waits.
_MAXW = 1
_orig_commit_and_lower = tile.TileContext._commit_and_lower


def _patched_commit_and_lower(self, inst, original_block, old_bb_map, bb_to_exit_bb):
    si = inst.sync_info
    if si is not None and si.on_wait is not None and len(si.on_wait) > _MAXW:
        waits = list(si.on_wait)
        for w in waits[:-_MAXW]:
            nop = self.nc.engines[inst.engine].nop(nofuse=True)
            nop.ins.sync_info = mybir.SyncInfo(on_wait=[w], on_update=[])
        inst.sync_info = mybir.SyncInfo(
            on_wait=waits[-_MAXW:], on_update=list(si.on_update or [])
        )
    _orig_commit_and_lower(self, inst, original_block, old_bb_map, bb_to_exit_bb)


def _patched_drain_and_barrier(self, tick_clock, wait_clock):
    nc = self.nc
    probe = nc.sync.nop(nofuse=True)
    wait_clock.add_sem_waits(probe.ins, ScopedClock({None: tick_clock.global_clock}))
    si = probe.ins.sync_info
    waits = list(si.on_wait) if si is not None else []
    if len(waits) > _MAXW:
        probe.ins.sync_info = mybir.SyncInfo(on_wait=waits[:_MAXW], on_update=[])
        for i in range(_MAXW, len(waits), _MAXW):
            extra = nc.sync.nop(nofuse=True)
            extra.ins.sync_info = mybir.SyncInfo(
                on_wait=waits[i : i + _MAXW], on_update=[]
            )
    nc.sync.drain()
    nc.all_engine_barrier()
    assert self.sems is not None
    popped = nc._tile_sem_poison_stack.pop()
    assert popped is self._sem_poison
    nc.clear_and_free_semaphores(list(self.sems.allocated().values()))
    nc.all_engine_barrier()


def _act_recip_lnexp(nc, out, in_):
    # 1/s computed as exp(-ln(s)) on the Scalar engine.  ln and exp live in
    # the same activation table (natural_log_exp_and_others) so this costs
    # no ACT_TABLE_LOAD swaps, unlike the Reciprocal table op; and the DVE
    # InstReciprocal runs at ~6.6 ns/element on one partition (125 us total
    # here).  Inputs are softmax sums in [~50, 4000]: ln/exp tables are
    # accurate to ~1e-4 there, well inside tolerance.
    nc.scalar.activation(out, in_, mybir.ActivationFunctionType.Ln)
    nc.scalar.activation(out, out, mybir.ActivationFunctionType.Exp, scale=-1.0)


def _install_patches():
    tile.TileContext._commit_and_lower = _patched_commit_and_lower
    tile.TileContext._drain_and_barrier = _patched_drain_and_barrier


# ---------------------------------------------------------------- host prep
def _relative_position_index():
    coords = np.stack(np.meshgrid(np.arange(WH), np.arange(WW), indexing="ij"))
    cf = coords.reshape(2, -1)
    rel = cf[:, :, None] - cf[:, None, :]
    rel = rel.transpose(1, 2, 0).astype(np.int64)
    rel[:, :, 0] += WH - 1
    rel[:, :, 1] += WW - 1
    rel[:, :, 0] *= 2 * WW - 1
    idx = np.zeros((N_TOK, N_TOK), dtype=np.int64)
    idx[1:, 1:] = rel.sum(-1)
    idx[0, 0:] = NUM_REL_DIST - 3
    idx[0:, 0] = NUM_REL_DIST - 2
    idx[0, 0] = NUM_REL_DIST - 1
    return idx


def _host_prepare(x, qkv_weight, q_bias, v_bias, rel_pos_bias_table, proj_weight,
                  proj_bias):
    wqk = qkv_weight[: 2 * DIM].astype(np.float32).copy()
    wqk[:DIM] *= SCALE
    wqkT = np.ascontiguousarray(wqk.T).astype(np.float16)  # [768, 1536]
    wvT = np.ascontiguousarray(qkv_weight[2 * DIM :].T).astype(np.float16)
    wprojT = np.ascontiguousarray(proj_weight.T).astype(np.float16)  # [in, out]
    qkb = np.ascontiguousarray(
        (q_bias.astype(np.float32) * SCALE).reshape(6, 128).T
    ).astype(np.float32)  # [128, 6]
    vb = v_bias.astype(np.float16)
    pb = proj_bias.astype(np.float32)

    idx = _relative_position_index()
    bias_qk = rel_pos_bias_table.astype(np.float32)[idx]  # [q, k, h]
    e = np.exp(bias_qk).transpose(1, 2, 0)  # [k, h, q]
    e = np.ascontiguousarray(e.reshape(N_TOK, NPAIR, 2 * N_TOK)).astype(np.float16)
    rpb0 = np.ascontiguousarray(e[:128])  # [128, 6, 394]
    rpb1 = np.ascontiguousarray(e[128:])  # [69, 6, 394]

    shared = dict(wqkT=wqkT, wvT=wvT, wprojT=wprojT, qkb=qkb, vbias=vb,
                  pbias=pb, rpb0=rpb0, rpb1=rpb1)
    in_maps = []
    for c in range(N_CORES):
        xc = x[c * B_LOC : (c + 1) * B_LOC].reshape(TOK, DIM).astype(np.float16)
        xT = np.ascontiguousarray(xc.T)  # [768, 1576]
        in_maps.append(dict(xT=xT, **shared))
    return in_maps


# ------------------------------------------------------------- device build
def build_nc(phases=4, sub=9):
    _install_patches()
    nc = bass.Bass("TRN2", target_bir_lowering=False, debug=False,
                   num_devices=N_CORES)

    xT = nc.dram_tensor("xT", [DIM, TOK], F16, kind="ExternalInput")
    wqkT = nc.dram_tensor("wqkT", [DIM, 2 * DIM], F16, kind="ExternalInput")
    wvT = nc.dram_tensor("wvT", [DIM, DIM], F16, kind="ExternalInput")
    wprojT = nc.dram_tensor("wprojT", [DIM, DIM], F16, kind="ExternalInput")
    qkb = nc.dram_tensor("qkb", [128, 6], F32, kind="ExternalInput")
    vbias = nc.dram_tensor("vbias", [DIM], F16, kind="ExternalInput")
    pbias = nc.dram_tensor("pbias", [DIM], F32, kind="ExternalInput")
    rpb0 = nc.dram_tensor("rpb0", [128, NPAIR, 2 * N_TOK], F16, kind="ExternalInput")
    rpb1 = nc.dram_tensor("rpb1", [69, NPAIR, 2 * N_TOK], F16, kind="ExternalInput")
    out = nc.dram_tensor("out", [TOK, DIM], F32, kind="ExternalOutput")

    def bcast_ap(handle, n):
        ap = handle.ap()
        return bass.AP(tensor=ap.tensor, offset=ap.offset,
                       ap=[[0, 128]] + list(ap.ap))

    with tile.TileContext(nc) as tc:
        with (
            tc.tile_pool(name="const", bufs=1) as const,
            tc.tile_pool(name="exp", bufs=6) as exp_pool,
            tc.tile_pool(name="attn", bufs=3) as attn_pool,
            tc.tile_pool(name="bc", bufs=6) as bc_pool,
            tc.tile_pool(name="outp", bufs=3) as out_pool,
            tc.tile_pool(name="rc", bufs=3) as rc_pool,
            tc.tile_pool(name="un", bufs=4) as un_pool,
            tc.tile_pool(name="dscr", bufs=6, space="DRAM") as dscr_pool,
            tc.tile_pool(name="psA", bufs=4, space="PSUM") as psum,
            tc.tile_pool(name="psB", bufs=2, space="PSUM") as psum_pv,
        ):
            def rep_rows(row_ap, n):
                # replicate a [1, F] SBUF row across n partitions (step-0
                # partition dim) -- DMA source AP
                return bass.AP(tensor=row_ap.tensor, offset=row_ap.offset,
                               ap=[[0, n]] + list(row_ap.ap)[1:])

            # ---- constants into SBUF
            xT_sb = const.tile([128, 6, TOK], F16, tag="xT")
            xT_r = xT.ap().rearrange("(a p) n -> p a n", p=128)
            wqkT_sb = const.tile([128, 6, 2 * DIM], F16, tag="wqkT")
            wqk_r = wqkT.ap().rearrange("(a p) n -> p a n", p=128)
            for k in range(6):
                nc.sync.dma_start(wqkT_sb[:, k, :], wqk_r[:, k, :])
                nc.sync.dma_start(xT_sb[:, k, :], xT_r[:, k, :])
            wvT_sb = const.tile([128, 6, DIM], F16, tag="wvT")
            nc.sync.dma_start(wvT_sb[:], wvT.ap().rearrange("(a p) n -> p a n", p=128))
            wprojT_sb = const.tile([128, 6, DIM], F16, tag="wprojT")
            nc.sync.dma_start(wprojT_sb[:], wprojT.ap().rearrange("(a p) n -> p a n", p=128))
            qkb_sb = const.tile([128, 6], F32, tag="qkb")
            nc.sync.dma_start(qkb_sb[:], qkb.ap())
            vb_sb = const.tile([128, DIM], F16, tag="vb")
            nc.gpsimd.dma_start(vb_sb[:], bcast_ap(vbias, DIM))
            pb_sb = const.tile([128, DIM], F32, tag="pb")
            nc.gpsimd.dma_start(pb_sb[:], bcast_ap(pbias, DIM))
            rpb0_sb = const.tile([128, NPAIR, 2 * N_TOK], F16, tag="rpb0")
            nc.sync.dma_start(rpb0_sb[:], rpb0.ap())
            rpb1_sb = const.tile([69, NPAIR, 2 * N_TOK], F16, tag="rpb1")
            nc.sync.dma_start(rpb1_sb[:], rpb1.ap())

            kT_sb = const.tile([128, 6, TOK], F16, tag="kT")
            # q in zero-padded head slots: slot (c, j) holds head 2c+j on
            # partitions 64j:64j+64, zeros elsewhere, so QK^T runs as a
            # plain K=128 matmul against the packed k chunk.  (Row-group
            # packed K=64 matmul pairs crash this runtime.)
            qz_sb = const.tile([128, 6, 2, TOK], F16, tag="qz")
            nc.gpsimd.memset(qz_sb[0:64, :, 1, :], 0.0)
            nc.gpsimd.memset(qz_sb[64:128, :, 0, :], 0.0)
            # v extended layout per (token-tile, pair):
            #   [0:64]=v_even [64]=1 | odd block (65+): [0:32]=0 [32]=1
            #   [33:64]=0 [64:128]=v_odd  -> odd sums land on psum row 32
            vext_sb = const.tile([128, 2 * B_LOC, NPAIR, 193], F16, tag="vext")
            nc.gpsimd.memset(vext_sb[:, :, :, 65:97], 0.0)
            nc.gpsimd.memset(vext_sb[:, :, :, 98:129], 0.0)
            nc.vector.memset(vext_sb[:, :, :, 64:65], 1.0)
            nc.vector.memset(vext_sb[:, :, :, 97:98], 1.0)

            # ---- phase 1: qkT [ch, tok] = Wqk' @ x^T (+ scaled q bias)
            NCH = 394  # 2 batches of queries per psum chunk
            for m in range(12):
                for nch in range(4):
                    ps = psum.tile([128, NCH], F32, tag="ps")
                    for k in range(6):
                        nc.tensor.matmul(
                            ps[:],
                            lhsT=wqkT_sb[:, k, m * 128 : (m + 1) * 128],
                            rhs=xT_sb[:, k, nch * NCH : (nch + 1) * NCH],
                            start=(k == 0), stop=(k == 5),
                        )
                    cols = slice(nch * NCH, (nch + 1) * NCH)
                    if m < 6:
                        nc.scalar.activation(
                            qz_sb[0:64, m, 0, cols], ps[0:64],
                            mybir.ActivationFunctionType.Identity,
                            bias=qkb_sb[0:64, m : m + 1],
                        )
                        nc.scalar.activation(
                            qz_sb[64:128, m, 1, cols], ps[64:128],
                            mybir.ActivationFunctionType.Identity,
                            bias=qkb_sb[64:128, m : m + 1],
                        )
                    else:
                        nc.scalar.activation(
                            kT_sb[:, m - 6, cols], ps[:],
                            mybir.ActivationFunctionType.Copy)

            # ---- phase 2: v (token-major) into vext (+ v bias)
            for bt in range(2 * B_LOC):
                b, t = divmod(bt, 2)
                tbase, tsz = KT[t]
                col0 = b * N_TOK + tbase
                for ncb in range(2):
                    ps = psum.tile([128, 384], F32, tag="ps")
                    for k in range(6):
                        nc.tensor.matmul(
                            ps[:tsz],
                            lhsT=xT_sb[:, k, col0 : col0 + tsz],
                            rhs=wvT_sb[:, k, ncb * 384 : (ncb + 1) * 384],
                            start=(k == 0), stop=(k == 5),
                        )
                    src = ps[:tsz].rearrange("p (c j d) -> p c j d", c=3, j=2)
                    vbv = vb_sb[:tsz, ncb * 384 : (ncb + 1) * 384].rearrange(
                        "p (c j d) -> p c j d", c=3, j=2)
                    pear = vext_sb[:tsz, bt, 3 * ncb : 3 * ncb + 3, :]
                    nc.vector.tensor_add(
                        out=pear[:, :, 0:64], in0=src[:, :, 0, :], in1=vbv[:, :, 0, :])
                    nc.vector.tensor_add(
                        out=pear[:, :, 129:193], in0=src[:, :, 1, :], in1=vbv[:, :, 1, :])

            # ---- phase 3: attention per (batch, pair-group of 2 head-pairs)
            # proj for batch b is emitted after batch b+1's attention
            # (1-deep software pipeline) so proj matmuls fill the
            # attention chain's dependency stalls.
            pending_proj = []

            def emit_proj(b, attn_sb):
                q0 = b * N_TOK
                for tbase, tsz in KT:
                    osb = out_pool.tile([128, DIM], F32, tag="osb")
                    for ncb in range(2):
                        ps = psum.tile([128, 384], F32, tag="ps")
                        for k in range(6):
                            nc.tensor.matmul(
                                ps[:tsz],
                                lhsT=attn_sb[:, k, tbase : tbase + tsz],
                                rhs=wprojT_sb[:, k, ncb * 384 : (ncb + 1) * 384],
                                start=(k == 0), stop=(k == 5),
                            )
                        nc.vector.tensor_add(
                            out=osb[:tsz, ncb * 384 : (ncb + 1) * 384],
                            in0=ps[:tsz],
                            in1=pb_sb[:tsz, ncb * 384 : (ncb + 1) * 384],
                        )
                    nc.sync.dma_start(
                        out.ap()[q0 + tbase : q0 + tbase + tsz, :], osb[:tsz])

            for b in range(B_LOC if phases >= 3 else 0):
                q0 = b * N_TOK
                attn_sb = attn_pool.tile([128, 6, N_TOK], F16, tag="attn")
                for g in range(NPAIR // 2):
                    # PV psum for 2 pairs; 256-stride keeps each matmul
                    # region inside one PSUM bank.
                    pvg = psum_pv.tile([128, 2, 2, 256], F32, tag="pvg")
                    for pig in (0, 1):
                        c = 2 * g + pig
                        # scores^T + exp + rel-pos multiplier, per key tile
                        ets = []
                        for t, (kbase, ksz) in enumerate(KT):
                            kcol = q0 + kbase
                            ps = psum.tile([128, 2 * N_TOK], F32, tag="ps")
                            nc.tensor.matmul(
                                ps[:ksz, :].rearrange("p (j q) -> p j q", j=2),
                                lhsT=kT_sb[:, c, kcol : kcol + ksz],
                                rhs=qz_sb[:, c, :, q0 : q0 + N_TOK],
                                start=True, stop=True,
                            )
                            et = exp_pool.tile([128, 2 * N_TOK], F16, tag="exp")
                            nc.scalar.activation(
                                et[:ksz], ps[:ksz], mybir.ActivationFunctionType.Exp)
                            rp = rpb0_sb if t == 0 else rpb1_sb
                            nc.vector.tensor_mul(et[:ksz], et[:ksz], rp[:ksz, c, :])
                            ets.append((et, ksz))

                        # PV (attention-out transposed + sums rows)
                        for j in ((0, 1) if sub >= 2 else ()):
                            outap = (pvg[0:65, pig, 0, 0:N_TOK] if j == 0
                                     else pvg[:, pig, 1, 0:N_TOK])
                            lo, hi = (0, 65) if j == 0 else (65, 193)
                            for t, (et, ksz) in enumerate(ets):
                                nc.tensor.matmul(
                                    outap,
                                    lhsT=vext_sb[:ksz, 2 * b + t, c, lo:hi],
                                    rhs=et[:ksz, j * N_TOK : (j + 1) * N_TOK],
                                    start=(t == 0), stop=(t == 1),
                                )

                    # softmax denominators sit at psum row 64 (even heads,
                    # j=0 block) and row 32 (odd heads, j=1 block); 1/s via
                    # ln+exp on ACT (same activation table as Exp).
                    recip = rc_pool.tile([128, 2, 2 * N_TOK], F32, tag="recip")
                    _act_recip_lnexp(
                        nc, recip[64:65, :, 0:N_TOK], pvg[64:65, :, 0, 0:N_TOK])
                    _act_recip_lnexp(
                        nc, recip[32:33, :, N_TOK : 2 * N_TOK],
                        pvg[32:33, :, 1, 0:N_TOK])
                    # bounce recip rows through DRAM (SBUF APs cannot have
                    # step-0 partition dims), then one broadcast-read per
                    # pair: partitions 0:64 <- even row, 64:128 <- odd row.
                    dscr = dscr_pool.tile([2, 2, N_TOK], F32, tag="dscr")
                    nc.sync.dma_start(dscr[0:1, :, :], recip[64:65, :, 0:N_TOK])
                    nc.sync.dma_start(
                        dscr[1:2, :, :], recip[32:33, :, N_TOK : 2 * N_TOK])
                    for pig in (0, 1):
                        c = 2 * g + pig
                        bc = bc_pool.tile([128, N_TOK], F32, tag="bc")
                        nc.gpsimd.dma_start(
                            bc[0:64, :], rep_rows(dscr[0:1, pig, :], 64))
                        nc.gpsimd.dma_start(
                            bc[64:128, :], rep_rows(dscr[1:2, pig, :], 64))
                        nc.vector.tensor_mul(
                            attn_sb[0:64, c, :], pvg[0:64, pig, 0, 0:N_TOK],
                            bc[0:64, :])
                        nc.vector.tensor_mul(
                            attn_sb[64:128, c, :], pvg[64:128, pig, 1, 0:N_TOK],
                            bc[64:128, :])

                # ---- delayed proj (previous batch)
                if phases >= 4:
                    pending_proj.append((b, attn_sb))
                    if len(pending_proj) > 1:
                        emit_proj(*pending_proj.pop(0))
            if phases >= 4:
                while pending_proj:
                    emit_proj(*pending_proj.pop(0))
            if phases < 4:
                # debug: dump some qkT into out so the output is written
                dbg = out_pool.tile([128, DIM], F32, tag="osb")
                nc.scalar.activation(dbg[:, 0:DIM], kT_sb[:, 0, 0:DIM],
                                     mybir.ActivationFunctionType.Copy)
                nc.vector.tensor_add(dbg[:], dbg[:], vext_sb[:, 0, 0, 0:1].to_broadcast([128, DIM]))
                for r in range(0, TOK, 128):
                    sz = min(128, TOK - r)
                    nc.sync.dma_start(out.ap()[r : r + sz, :], dbg[:sz])
    return nc


_NC_CACHE = None


def _get_nc():
    global _NC_CACHE
    if _NC_CACHE is None:
        _NC_CACHE = build_nc()
    return _NC_CACHE


def _execute(inputs, trace=False):
    in_maps = _host_prepare(**inputs)
    nc = _get_nc()
    res = run_bass_kernel_spmd(nc, in_maps, core_ids=list(range(N_CORES)),
                               trace=trace)
    outs = [res.results[c]["out"].reshape(B_LOC, N_TOK, DIM) for c in range(N_CORES)]
    return np.concatenate(outs, axis=0), res


def kernel(**inputs) -> np.ndarray:
    out, _ = _execute(inputs, trace=False)
    return out
